# revision 1
# baseline (speedup 1.0000x reference)
"""Bidirectional Mamba block (in_proj -> depthwise causal conv -> SiLU ->
forward+backward S6 selective scan -> gated combine -> out_proj) as a
Trainium2 Bass/Tile SPMD kernel over 8 NeuronCores.

Sharding: tensor-parallel over d_inner (256 channels per core). Two small
collectives: AllReduce of dbc partials per direction, and a chunked bf16
ReduceScatter of the out-projection partials.

This revision runs the whole matmul pipeline in bf16 (4x cheaper
LDWEIGHTS, 2x DVE element throughput where both operands are 16-bit),
uses the ACT Softplus/Silu table functions, keeps all elementwise work
off GpSimd (it serializes against DVE on the shared SBUF port), folds
the u*D and y_fwd terms into the backward y PSUM accumulation via
diagonal/identity stationaries, and overlaps the output ReduceScatter
with the out-projection by chunking it over token blocks.
"""

import os
import sys

for _p in ("/opt/trn_rl_repo", "/root/.axon_site/_ro/trn_rl_repo"):
    if os.path.isdir(_p) and _p not in sys.path:
        sys.path.append(_p)

from dataclasses import dataclass

import ml_dtypes
import numpy as np

import concourse.bass as bass
import concourse.mybir as mybir
import concourse.tile as tile
from concourse import bacc

DT = mybir.dt.float32
BF = mybir.dt.bfloat16
AF = mybir.ActivationFunctionType
OP = mybir.AluOpType


# --------------------------------------------------------------------------
# Custom DVE op: interleaved affine scan (hand-written uop program).
#
# Over a [P, N] stream, computes TWO independent first-order affine
# recurrences interleaved along the free dim at 1 element/cycle (the stock
# TENSOR_TENSOR_SCAN needs ~2.15 cycles/element for a single sequence):
#     state[t%2] = fp32(in0[:,t] * state[t%2]) + in1[:,t];  out = state
# Even elements keep their state in datapath block 1's a-flop, odd elements
# in the b-flop; block 0 reads its own stream's flop via the spatial-
# backward path (NEXT_ALU_OUT_A/B), so correctness is stall-independent.
# --------------------------------------------------------------------------

ISCAN_NAME = "ANT_ISCAN_MAMBA"


def _iscan_mk_uop(seed: bool, flop: str, nxt: int):
    from concourse.dve_uop import (
        ENABLE, AluInp, AluOp, InpSel, OutPath, OutSel, Trigger, UopConfig,
    )
    u = UopConfig()
    u.enable_input(InpSel.SRC_0, 1)      # dA   -> block0 PREV_DELAY_0
    u.enable_input(InpSel.SRC_1, 2)      # dBu  -> block0 PREV_DELAY_1
    if seed:
        u.enable_input(InpSel.ZERO, 3)   # 0.0  -> block0 PREV_DELAY_2
        state_src = AluInp.PREV_DELAY_2
    else:
        state_src = (AluInp.NEXT_ALU_OUT_A if flop == "a"
                     else AluInp.NEXT_ALU_OUT_B)
    blocks = u.datapath_config
    blocks[0].enable_alu(AluOp.MULTIPLY, AluInp.PREV_DELAY_0, state_src)
    blocks[0].pass_through_delay(1)      # carry dBu to block 1
    blocks[1].enable_alu(AluOp.ADD, AluInp.PREV_ALU_OUT, AluInp.PREV_DELAY_1)
    if flop == "a":
        blocks[1].alu_out_a_enable = ENABLE
    else:
        blocks[1].alu_out_b_enable = ENABLE
    for k in range(2, 8):
        blocks[k].pass_through_alu()
    u.enable_output(OutSel.ALU_OUT, OutPath.WR0_LO)
    u.require_inp0 = ENABLE
    u.require_inp1 = ENABLE
    u.repeat_count = 1
    u.trigger = (Trigger.SRC_TENSOR_DONE, Trigger.COUNT, Trigger.NONE)
    u.next_uop = (0, nxt, 0)
    return u


class _IscanOp:
    """Duck-typed stand-in for dve_ops.DveOp carrying a hand-written uop
    program (the documented escape hatch around Spec/lower())."""

    name = ISCAN_NAME
    subdim = False
    perf_en: dict = {}
    spec = None

    def compile(self, ver):
        import concourse.dve_ops as dve_ops
        from concourse.dve_uop import DveOpSpec
        assert ver == "v3", "interleaved scan validated for TRN2 only"
        s = DveOpSpec(
            name=self.name,
            opcode=dve_ops.get_dve_sub_opcode(self.name),
            uops=[
                _iscan_mk_uop(seed=True, flop="a", nxt=1),
                _iscan_mk_uop(seed=True, flop="b", nxt=2),
                _iscan_mk_uop(seed=False, flop="a", nxt=3),
                _iscan_mk_uop(seed=False, flop="b", nxt=2),
            ],
            rd1_en=True,
        )
        s.validate(ver)
        return s


def _iscan_register():
    import concourse.dve_ops as dve_ops
    for op in dve_ops.OPS:
        if op.name == ISCAN_NAME:
            return op
    op = _IscanOp()
    dve_ops.OPS.append(op)
    dve_ops._SUB_OPCODE_FOR_NAME[ISCAN_NAME] = (
        dve_ops._CUSTOM_DVE_ROW_BASE + len(dve_ops.OPS) - 1
    )
    assert dve_ops._SUB_OPCODE_FOR_NAME[ISCAN_NAME] < 0x20
    return op


def iscan(vec_engine, out, in0, in1):
    """out[:, t] = in0[:, t] * state[t%2] + in1[:, t]  (per partition)."""
    op = _iscan_register()
    return vec_engine._custom_dve(op, out=out, in0=in0, in1=in1)


@dataclass(frozen=True)
class Cfg:
    n_cores: int = 8
    B: int = 2
    L: int = 1024
    M: int = 1024      # d_model
    DI: int = 2048     # d_inner
    N: int = 16        # d_state
    R: int = 64        # dt_rank
    KC: int = 4        # conv kernel

    @property
    def DC(self):  # channels per core
        return self.DI // self.n_cores

    @property
    def TOK(self):
        return self.B * self.L

    @property
    def P_CH(self):
        return min(128, self.DC)

    @property
    def CHT(self):  # channel tiles per core
        return self.DC // self.P_CH

    @property
    def NT(self):  # scan tiles per (dir): 8 channels each
        return self.DC // 8

    @property
    def TPC(self):  # scan tiles per channel tile
        return self.P_CH // 8

    @property
    def FCH(self):
        return min(512, self.L)

    @property
    def E(self):
        return self.R + 2 * self.N

    def check(self):
        assert self.DC % 8 == 0 and self.DC % self.P_CH == 0
        assert self.M % 128 == 0
        assert self.TOK % 128 == 0 and self.TOK % self.FCH == 0
        assert self.N == 16


FULL = Cfg()


def build_consts(cfg: Cfg):
    P = 128
    ident = np.eye(P, dtype=np.float32)
    # R_all[:, jj, :]: out[p] = src[8*jj + p//16]  (delta/w replication)
    r_all = np.zeros((cfg.P_CH, cfg.TPC, P), np.float32)
    for jj in range(cfg.TPC):
        for p in range(P):
            r_all[8 * jj + p // 16, jj, p] = 1.0
    # T_sel[:, which, :]: out[p] = src[16*which + p%16]  (B/C replication)
    t_sel = np.zeros((2 * cfg.N, 2, P), np.float32)
    for which in range(2):
        for p in range(P):
            t_sel[cfg.N * which + p % 16, which, p] = 1.0
    # S_all[:, jj, :]: reduce groups of 16 partitions into channel 8*jj+p//16
    s_all = np.zeros((P, cfg.TPC, cfg.P_CH), np.float32)
    for jj in range(cfg.TPC):
        for p in range(P):
            s_all[p, jj, 8 * jj + p // 16] = 1.0
    return ident, r_all, t_sel, s_all


def build_program(cfg: Cfg) -> bass.Bass:
    cfg.check()
    P = 128
    TOK, L, M, B = cfg.TOK, cfg.L, cfg.M, cfg.B
    DC, CHT, P_CH, NT, TPC, FCH = (cfg.DC, cfg.CHT, cfg.P_CH, cfg.NT,
                                   cfg.TPC, cfg.FCH)
    MT = M // P
    TBT = TOK // P
    NFC = TOK // FCH
    E, R, N, KC = cfg.E, cfg.R, cfg.N, cfg.KC

    nc = bacc.Bacc(
        "TRN2", target_bir_lowering=False, debug=False, num_devices=cfg.n_cores
    )

    # ---- kernel I/O ----
    x_d = nc.dram_tensor("x", [TOK, M], BF, kind="ExternalInput")
    winT_d = nc.dram_tensor("winT", [M, 2 * DC], BF, kind="ExternalInput")
    wconv_d = nc.dram_tensor("wconv", [P, CHT * KC], DT, kind="ExternalInput")
    bconv_d = nc.dram_tensor("bconv", [P, CHT], DT, kind="ExternalInput")
    wxT_d = {d: nc.dram_tensor(f"wx{d}T", [DC, E], BF, kind="ExternalInput")
             for d in "fb"}
    wdtT_d = {d: nc.dram_tensor(f"wdt{d}T", [R, DC], BF, kind="ExternalInput")
              for d in "fb"}
    bdt_d = {d: nc.dram_tensor(f"bdt{d}", [P, CHT], DT, kind="ExternalInput")
             for d in "fb"}
    acol_d = {d: nc.dram_tensor(f"acol{d}", [P, NT], DT, kind="ExternalInput")
              for d in "fb"}
    dsd_d = nc.dram_tensor("dsd", [P, CHT * P], BF, kind="ExternalInput")
    woutT_d = nc.dram_tensor("woutT", [cfg.DI, M], BF, kind="ExternalInput")
    ident_d = nc.dram_tensor("ident", [P, P], BF, kind="ExternalInput")
    rall_d = nc.dram_tensor("rall", [P_CH, TPC * P], BF, kind="ExternalInput")
    tsel_d = nc.dram_tensor("tsel", [2 * N, 2 * P], BF, kind="ExternalInput")
    sall_d = nc.dram_tensor("sall", [P, TPC * P_CH], BF, kind="ExternalInput")

    out_d = nc.dram_tensor("out_rs", [TOK // cfg.n_cores, M], DT,
                           kind="ExternalOutput")
    warm_d = nc.dram_tensor("warm", [P, 1], DT, kind="ExternalOutput")

    rg = [list(range(cfg.n_cores))]
    cc_space = "Shared" if cfg.n_cores > 4 else "Local"
    RSC = 4                       # ReduceScatter chunks
    TPB_RS = TBT // RSC           # token blocks per RS chunk
    RTOK = TOK // cfg.n_cores     # tokens per core after RS

    with tile.TileContext(nc) as tc:
        with tc.tile_pool(name="persist", bufs=1) as pp, \
             tc.tile_pool(name="dram", bufs=1, space="DRAM") as dp:

            # ---------- persistent SBUF ----------
            ident_s = pp.tile([P, P], BF)
            nc.sync.dma_start(ident_s[:], ident_d.ap())
            # PE warm-up burst: ~3.5us of back-to-back matmuls releases the
            # HAM clock gate (cold PE runs at 1.2 GHz) before in_proj
            with tc.tile_pool(name="warm", bufs=1) as wup, \
                 tc.tile_pool(name="warm_ps", bufs=1, space="PSUM") as wpp:
                wu_ps = wpp.tile([P, P], DT, tag="wu", bufs=2, name="wu")
                for _ in range(28):
                    nc.tensor.matmul(wu_ps[:], ident_s[:], ident_s[:],
                                     start=True, stop=True)
                wu_sb = wup.tile([P, 1], DT, name="wu_sb")
                nc.scalar.copy(wu_sb[:], wu_ps[:, :1])
                nc.sync.dma_start(warm_d.ap(), wu_sb[:])
            rall_s = pp.tile([P_CH, TPC, P], BF)
            nc.sync.dma_start(rall_s[:], rall_d.ap().rearrange(
                "k (a b) -> k a b", a=TPC))
            tsel_s = pp.tile([2 * N, 2, P], BF)
            nc.sync.dma_start(tsel_s[:], tsel_d.ap().rearrange(
                "k (a b) -> k a b", a=2))
            sall_s = pp.tile([P, TPC, P_CH], BF)
            nc.sync.dma_start(sall_s[:], sall_d.ap().rearrange(
                "p (a b) -> p a b", a=TPC))
            wconv_s = pp.tile([P, CHT, KC], DT)
            nc.sync.dma_start(wconv_s[:], wconv_d.ap().rearrange(
                "p (c k) -> p c k", c=CHT))
            bconv_s = pp.tile([P, CHT], DT)
            nc.sync.dma_start(bconv_s[:], bconv_d.ap())
            dsd_s = pp.tile([P, CHT, P], BF)
            nc.sync.dma_start(dsd_s[:], dsd_d.ap().rearrange(
                "p (c q) -> p c q", c=CHT))
            wx_s, wdt_s, bdt_s, acol_s = {}, {}, {}, {}
            for d in "fb":
                wx_s[d] = pp.tile([P_CH, CHT, E], BF, name=f"wx{d}_s")
                nc.sync.dma_start(wx_s[d][:], wxT_d[d].ap().rearrange(
                    "(c p) e -> p c e", p=P_CH))
                wdt_s[d] = pp.tile([R, DC], BF, name=f"wdt{d}_s")
                nc.sync.dma_start(wdt_s[d][:], wdtT_d[d].ap())
                bdt_s[d] = pp.tile([P, CHT], DT, name=f"bdt{d}_s")
                nc.sync.dma_start(bdt_s[d][:], bdt_d[d].ap())
                acol_s[d] = pp.tile([P, NT], DT, name=f"acol{d}_s")
                nc.sync.dma_start(acol_s[d][:], acol_d[d].ap())
            win_s = pp.tile([P, MT, 2 * DC], BF)
            nc.sync.dma_start(win_s[:], winT_d.ap().rearrange(
                "(a p) c -> p a c", p=P))


            u_c = [pp.tile([P_CH, TOK], BF, name=f"u_c{c}") for c in range(CHT)]
            sres = [pp.tile([P_CH, TOK], BF, name=f"sres{c}")
                    for c in range(CHT)]
            y_f = [pp.tile([P_CH, TOK], BF, name=f"y_f{c}") for c in range(CHT)]
            yfin = [pp.tile([P_CH, TOK], BF, name=f"yfin{c}")
                    for c in range(CHT)]
            delta_s = {d: [pp.tile([P_CH, TOK], BF, name=f"delta_{d}{c}")
                           for c in range(CHT)] for d in "fb"}
            w_sd = {d: [pp.tile([P_CH, TOK], BF, name=f"w_{d}{c}")
                        for c in range(CHT)] for d in "fb"}
            brep_s = {d: pp.tile([P, TOK], BF, name=f"brep{d}") for d in "fb"}
            crep_s = {d: pp.tile([P, TOK], BF, name=f"crep{d}") for d in "fb"}

            dbc_part = {d: [dp.tile([E, L], DT, name=f"dbc_part_{d}{hf}")
                            for hf in range(2)] for d in "fb"}
            dbc_red = {d: [dp.tile([E, L], DT, addr_space=cc_space,
                                   name=f"dbc_red_{d}{hf}")
                           for hf in range(2)] for d in "fb"}
            y_dram = dp.tile([TOK, DC], BF, name="y_dram")
            y_a2a = dp.tile([TOK, DC], BF, name="y_a2a")


            # ---------- phase 1-3: in_proj, conv, silu, dbc, res ----------
            with tc.tile_pool(name="proj", bufs=1) as jp, \
                 tc.tile_pool(name="proj_ps", bufs=1, space="PSUM") as jpp:
                xT_s = jp.tile([P, MT, TOK], BF, name="xT_s")
                PAD = 2 * (KC - 1)
                upad = [jp.tile([P_CH, PAD + TOK], BF, name=f"upad{c}")
                        for c in range(CHT)]
                for c in range(CHT):
                    nc.gpsimd.memset(upad[c][:, :PAD], 0.0)

                # fc-major pipeline: in_proj -> conv -> dbc partials per
                # token chunk, so the dbc AllReduces (issued per half) can
                # launch as early as possible.
                def conv_chunk(c, f0):
                    acc = None
                    for k in range(KC):
                        nxt = cp.tile([P_CH, FCH], BF, tag="cacc",
                                      bufs=2, name="cacc")
                        tap = upad[c][:, 2 * k + f0:2 * k + f0 + FCH]
                        wk = wconv_s[:P_CH, c, k:k + 1]
                        if acc is None:
                            nc.vector.tensor_scalar(
                                nxt[:], tap, wk,
                                bconv_s[:P_CH, c:c + 1],
                                OP.mult, OP.add)
                        else:
                            nc.vector.scalar_tensor_tensor(
                                nxt[:], tap, wk, acc[:],
                                OP.mult, OP.add)
                        acc = nxt
                    nc.scalar.activation(u_c[c][:, f0:f0 + FCH], acc[:],
                                         AF.Silu)

                TPG = 4
                with tc.tile_pool(name="conv", bufs=1) as cp:
                    for fc in range(NFC):
                        f0 = fc * FCH
                        # transpose this token quarter of x (tb = pi blocks)
                        for tb in range(fc * TBT // NFC,
                                        (fc + 1) * TBT // NFC):
                            xb = jp.tile([P, M], BF, tag="xb", bufs=3,
                                         name="xb")
                            nc.sync.dma_start(
                                xb[:], x_d.ap()[tb * P:(tb + 1) * P, :])
                            for mg in range(MT // TPG):
                                tp = jpp.tile([P, TPG, P], BF, tag="tp",
                                              bufs=2, name="tp")
                                for k in range(TPG):
                                    mt = mg * TPG + k
                                    nc.tensor.transpose(
                                        tp[:, k, :],
                                        xb[:, mt * P:(mt + 1) * P],
                                        ident_s[:])
                                nc.scalar.copy(
                                    xT_s[:, mg * TPG:(mg + 1) * TPG,
                                         tb * P:(tb + 1) * P], tp[:])
                        for c in range(CHT):
                            ups = jpp.tile([P_CH, FCH], DT, tag="mm", bufs=4,
                                           name="ups")
                            for kt in range(MT):
                                nc.tensor.matmul(
                                    ups[:],
                                    win_s[:, kt, c * P_CH:(c + 1) * P_CH],
                                    xT_s[:, kt, f0:f0 + FCH],
                                    start=(kt == 0), stop=(kt == MT - 1))
                            nc.scalar.copy(
                                upad[c][:, PAD + f0:PAD + f0 + FCH], ups[:])
                            conv_chunk(c, f0)
                        bps = jpp.tile([E, FCH], DT, tag="mm", bufs=4,
                                       name="bps")
                        for c in range(CHT):
                            nc.tensor.matmul(
                                bps[:],
                                wx_s["f"][:, c, :],
                                u_c[c][:, f0:f0 + FCH],
                                start=(c == 0), stop=(c == CHT - 1))
                        bst = jp.tile([E, FCH], DT, tag="bst", bufs=3,
                                      name="bst")
                        nc.scalar.copy(bst[:], bps[:])
                        o0 = f0 - (fc // 2) * L
                        nc.sync.dma_start(
                            dbc_part["f"][fc // 2][:, o0:o0 + FCH], bst[:])
                        if fc % 2 == 1:
                            nc.gpsimd.collective_compute(
                                "AllReduce", OP.add, replica_groups=rg,
                                ins=[dbc_part["f"][fc // 2].opt()],
                                outs=[dbc_red["f"][fc // 2].opt()])
                    for fc in range(NFC):
                        f0 = fc * FCH
                        bps = jpp.tile([E, FCH], DT, tag="mm", bufs=4,
                                       name="bps")
                        for c in range(CHT):
                            nc.tensor.matmul(
                                bps[:],
                                wx_s["b"][:, c, :],
                                u_c[c][:, f0:f0 + FCH],
                                start=(c == 0), stop=(c == CHT - 1))
                        bst = jp.tile([E, FCH], DT, tag="bst", bufs=3,
                                      name="bst")
                        nc.scalar.copy(bst[:], bps[:])
                        o0 = f0 - (fc // 2) * L
                        nc.sync.dma_start(
                            dbc_part["b"][fc // 2][:, o0:o0 + FCH], bst[:])
                        if fc % 2 == 1:
                            nc.gpsimd.collective_compute(
                                "AllReduce", OP.add, replica_groups=rg,
                                ins=[dbc_part["b"][fc // 2].opt()],
                                outs=[dbc_red["b"][fc // 2].opt()])

                # res projection -> silu gate (overlaps the AllReduces)
                for c in range(CHT):
                    for fc in range(NFC):
                        f0 = fc * FCH
                        rps = jpp.tile([P_CH, FCH], DT, tag="mm", bufs=4,
                                       name="rps")
                        for kt in range(MT):
                            nc.tensor.matmul(
                                rps[:],
                                win_s[:, kt, DC + c * P_CH:DC + (c + 1) * P_CH],
                                xT_s[:, kt, f0:f0 + FCH],
                                start=(kt == 0), stop=(kt == MT - 1))
                        nc.scalar.activation(sres[c][:, f0:f0 + FCH],
                                             rps[:], AF.Silu)


            # out-projection weights pool opened before the scans so the
            # 4MB full-W_out load overlaps the scan phase
            with tc.tile_pool(name="owt", bufs=1) as owp:
                wout_all = owp.tile([P_CH, cfg.n_cores * CHT, M], BF,
                                    name="wout_all")
                nc.sync.dma_start(wout_all[:], woutT_d.ap().rearrange(
                    "(k p) m -> p k m", p=P_CH))
                # ---------- phase 4: delta prep + scans ----------
                with tc.tile_pool(name="scan_sb", bufs=1) as sp, \
                     tc.tile_pool(name="scan_ps", bufs=1, space="PSUM") as spp:
                    def prep_dir(d):
                        # per-half: each half's work gates only on its own
                        # (smaller, faster) AllReduce
                        for hf in range(2):
                            h0 = hf * L
                            sl = slice(h0, h0 + L)
                            dtf_t = sp.tile([R, L], DT, tag="dtf", bufs=2,
                                            name="dtf")
                            nc.sync.dma_start(dtf_t[:], dbc_red[d][hf][:R, :])
                            dtb_t = sp.tile([R, L], BF, tag="dtb", bufs=2,
                                            name="dtb")
                            nc.vector.tensor_copy(dtb_t[:], dtf_t[:])
                            bcf_t = sp.tile([2 * N, L], DT, tag="bcf", bufs=2,
                                            name="bcf")
                            nc.sync.dma_start(bcf_t[:], dbc_red[d][hf][R:, :])
                            bcb_t = sp.tile([2 * N, L], BF, tag="bcb", bufs=2,
                                            name="bcb")
                            nc.vector.tensor_copy(bcb_t[:], bcf_t[:])
                            for c in range(CHT):
                                spt = sp.tile([P_CH, L], DT, tag="spt",
                                              bufs=2, name="spt")
                                for q in range(L // FCH):
                                    f0 = q * FCH
                                    dps = spp.tile([P_CH, FCH], DT, tag="wp",
                                                   bufs=1, name="dps")
                                    nc.tensor.matmul(
                                        dps[:],
                                        wdt_s[d][:, c * P_CH:(c + 1) * P_CH],
                                        dtb_t[:, f0:f0 + FCH],
                                        start=True, stop=True)
                                    # softplus(z+bdt) = ln(1 + exp(z+bdt))
                                    nc.scalar.activation(
                                        spt[:, f0:f0 + FCH], dps[:], AF.Exp,
                                        bias=bdt_s[d][:P_CH, c:c + 1])
                                nc.scalar.activation(
                                    delta_s[d][c][:, sl], spt[:],
                                    AF.Ln, bias=1.0)
                                nc.vector.tensor_tensor(
                                    w_sd[d][c][:, sl],
                                    delta_s[d][c][:, sl],
                                    u_c[c][:, sl], OP.mult)
                            for which, rep in ((0, brep_s[d]),
                                               (1, crep_s[d])):
                                for q in range(L // FCH):
                                    f0 = q * FCH
                                    rp2 = spp.tile([P, FCH], DT, tag="wp",
                                                   bufs=1, name="rp2")
                                    nc.tensor.matmul(
                                        rp2[:], tsel_s[:, which, :],
                                        bcb_t[:, f0:f0 + FCH],
                                        start=True, stop=True)
                                    nc.scalar.copy(
                                        rep[:, h0 + f0:h0 + f0 + FCH],
                                        rp2[:])

                    y_ps_holder = [None]

                    def scan_tile(d, j):
                        c = j // TPC
                        jj = j % TPC
                        rsel = rall_s[:, jj, :]
                        dA = sp.tile([P, TOK], BF, tag="dA", bufs=2,
                                     name="dA")
                        dBu = sp.tile([P, TOK], BF, tag="dBu", bufs=2,
                                      name="dBu")
                        for hf in range(2):
                            o = hf * L
                            wpc = spp.tile([P, L], DT, tag="wp", bufs=1,
                                           name="wpc")
                            for q in range(L // FCH):
                                nc.tensor.matmul(
                                    wpc[:, q * FCH:(q + 1) * FCH], rsel,
                                    w_sd[d][c][:, o + q * FCH:
                                               o + (q + 1) * FCH],
                                    start=True, stop=True)
                            if hf == 0:
                                # evacuate via ACT so the dBu multiply runs
                                # at DVE 2x on bf16 (balances ACT vs DVE)
                                wsb = sp.tile([P, L], BF, tag="wsb", bufs=2,
                                              name="wsb")
                                nc.scalar.copy(wsb[:], wpc[:])
                                nc.vector.tensor_tensor(
                                    dBu[:, o:o + L], wsb[:],
                                    brep_s[d][:, o:o + L], OP.mult)
                            else:
                                nc.vector.tensor_tensor(
                                    dBu[:, o:o + L], wpc[:],
                                    brep_s[d][:, o:o + L], OP.mult)
                            dpc = spp.tile([P, L], DT, tag="dp", bufs=1,
                                           name="dpc")
                            for q in range(L // FCH):
                                nc.tensor.matmul(
                                    dpc[:, q * FCH:(q + 1) * FCH], rsel,
                                    delta_s[d][c][:, o + q * FCH:
                                                  o + (q + 1) * FCH],
                                    start=True, stop=True)
                            nc.scalar.activation(
                                dA[:, o:o + L], dpc[:], AF.Exp,
                                scale=acol_s[d][:, j:j + 1])
                        h = sp.tile([P, TOK], BF, tag="h", bufs=2, name="h")
                        if d == "f":
                            iscan(nc.vector, h[:], dA[:], dBu[:])
                        else:
                            iscan(nc.vector, h[:, ::-1], dA[:, ::-1],
                                  dBu[:, ::-1])
                        hC = sp.tile([P, TOK], BF, tag="hC", bufs=2,
                                     name="hC")
                        nc.vector.tensor_tensor(hC[:], h[:], crep_s[d][:],
                                                OP.mult)
                        if jj == 0:
                            y_ps_holder[0] = [
                                spp.tile([P_CH, L], DT, tag=f"y{b}",
                                         bufs=1, name=f"y_ps{b}")
                                for b in range(B)]
                            if d == "b":
                                # seed with u*(fD+bD) + y_fwd
                                for b in range(B):
                                    for q in range(L // FCH):
                                        o = b * L + q * FCH
                                        oo = q * FCH
                                        nc.tensor.matmul(
                                            y_ps_holder[0][b][:, oo:oo + FCH],
                                            dsd_s[:, c, :],
                                            u_c[c][:, o:o + FCH],
                                            start=True, stop=False,
                                            skip_group_check=True)
                                        nc.tensor.matmul(
                                            y_ps_holder[0][b][:, oo:oo + FCH],
                                            ident_s[:],
                                            y_f[c][:, o:o + FCH],
                                            start=False, stop=False,
                                            skip_group_check=True)
                        y_ps = y_ps_holder[0]
                        for b in range(B):
                            for q in range(L // FCH):
                                o = b * L + q * FCH
                                oo = q * FCH
                                nc.tensor.matmul(
                                    y_ps[b][:, oo:oo + FCH],
                                    sall_s[:, jj, :],
                                    hC[:, o:o + FCH],
                                    start=(jj == 0 and d == "f"),
                                    stop=(jj == TPC - 1),
                                    skip_group_check=True)
                        if jj != TPC - 1:
                            return
                        for b in range(B):
                            sl = slice(b * L, (b + 1) * L)
                            if d == "f":
                                nc.scalar.copy(y_f[c][:, sl], y_ps[b][:])
                            else:
                                nc.vector.tensor_tensor(
                                    yfin[c][:, sl], y_ps[b][:],
                                    sres[c][:, sl], OP.mult)

                    def y_transpose(c, tb):
                        # stream one token block of gated y out as [tok, ch]
                        # for the AllToAll
                        tpy = spp.tile([P, P], BF, tag="dp", bufs=1,
                                       name="tpy")
                        nc.tensor.transpose(
                            tpy[:], yfin[c][:, tb * P:(tb + 1) * P],
                            ident_s[:])
                        ytp = sp.tile([P, P], BF, tag="ytp", bufs=2,
                                      name="ytp")
                        nc.scalar.copy(ytp[:], tpy[:])
                        nc.sync.dma_start(
                            y_dram[tb * P:(tb + 1) * P,
                                   c * P_CH:(c + 1) * P_CH], ytp[:])

                    prep_dir("f")
                    for j in range(NT):
                        scan_tile("f", j)
                    prep_dir("b")
                    for j in range(NT):
                        scan_tile("b", j)
                        if j >= TPC:
                            # ctile 0's y is final; spread its transposes
                            y_transpose(0, j - TPC)
                    for tb in range(TBT):
                        y_transpose(1, tb)

                # ---------- phase 6: y AllToAll + local out_proj ----------
                # transpose y to [tok, ch], exchange token windows between the
                # cores (pure data movement -- the CC cores reduce bf16 in slow
                # firmware, so AllToAll(1MB) beats ReduceScatter(4.2MB) by a lot),
                # transpose back, and contract all 2048 channels locally for this
                # core's 256-token window.
                with tc.tile_pool(name="out_ps", bufs=1, space="PSUM") as opp, \
                     tc.tile_pool(name="out_sb", bufs=1) as osp:
                    nc.gpsimd.collective_compute(
                        "AllToAll", OP.bypass, replica_groups=rg,
                        ins=[y_dram.opt()], outs=[y_a2a.opt()])
                    # load back + transpose to [ch, tok]
                    yall = [osp.tile([P_CH, cfg.n_cores, RTOK], BF,
                                     name=f"yall{c}") for c in range(CHT)]
                    for i in range(cfg.n_cores):
                        for t2 in range(RTOK // P):
                            la = osp.tile([P, DC], BF, tag="la", bufs=4,
                                          name="la")
                            nc.sync.dma_start(
                                la[:],
                                y_a2a[i * RTOK + t2 * P:i * RTOK + (t2 + 1) * P,
                                      :])
                            tp2 = opp.tile([P_CH, CHT, P], BF, tag="ytp",
                                           bufs=2, name="tp2")
                            for c in range(CHT):
                                nc.tensor.transpose(
                                    tp2[:, c, :], la[:, c * P_CH:(c + 1) * P_CH],
                                    ident_s[:])
                            for c in range(CHT):
                                nc.scalar.copy(
                                    yall[c][:, i, t2 * P:(t2 + 1) * P],
                                    tp2[:, c, :])
                    # local out_proj over all 2048 channels
                    MFC = min(512, M)
                    for t2 in range(RTOK // P):
                        ops = opp.tile([P, M], DT, tag="out", bufs=2, name="ops")
                        for mc in range(M // MFC):
                            o = mc * MFC
                            nmm = cfg.n_cores * CHT
                            k = 0
                            for i in range(cfg.n_cores):
                                for c in range(CHT):
                                    nc.tensor.matmul(
                                        ops[:, o:o + MFC],
                                        yall[c][:, i, t2 * P:(t2 + 1) * P],
                                        wout_all[:, i * CHT + c, o:o + MFC],
                                        start=(k == 0), stop=(k == nmm - 1))
                                    k += 1
                        ost = osp.tile([P, M], DT, tag="ost", bufs=2, name="ost")
                        nc.scalar.copy(ost[:], ops[:])
                        nc.sync.dma_start(out_d.ap()[t2 * P:(t2 + 1) * P, :],
                                          ost[:])

    nc.compile()
    return nc


# --------------------------------------------------------------------------
# host side
# --------------------------------------------------------------------------

def host_prep(cfg: Cfg, inputs: dict) -> list[dict]:
    P = 128
    f32 = np.float32
    bf16 = ml_dtypes.bfloat16

    def g(name):
        return np.asarray(inputs[name], f32)

    x = g("x").reshape(cfg.TOK, cfg.M)
    # pi (interleaved) token order: row 2q+b = (batch b, token q)
    x = np.ascontiguousarray(
        x.reshape(cfg.B, cfg.L, cfg.M).transpose(1, 0, 2)
        .reshape(cfg.TOK, cfg.M)).astype(bf16)
    W_in = g("W_in")
    W_conv = g("W_conv").reshape(cfg.DI, cfg.KC)
    b_conv = g("b_conv")
    W_out = g("W_out")
    ident, r_all, t_sel, s_all = build_consts(cfg)
    sall_flat = s_all.reshape(P, cfg.TPC * cfg.P_CH)
    rall_flat = r_all.reshape(cfg.P_CH, cfg.TPC * P)
    tsel_flat = t_sel.reshape(2 * cfg.N, 2 * P)

    per = {}
    for d in "fb":
        per[d] = dict(
            A=-np.exp(g(d + "A_log")),
            D=g(d + "D"),
            Wx=g(d + "Wx"),
            Wdt=g(d + "Wdt"),
            bdt=g(d + "bdt"),
        )

    def col_layout(v):
        return np.ascontiguousarray(
            v.reshape(cfg.CHT, cfg.P_CH).T.astype(f32))

    def pad_p(a):
        if a.shape[0] == P:
            return np.ascontiguousarray(a.astype(f32))
        out = np.zeros((P,) + a.shape[1:], f32)
        out[:a.shape[0]] = a
        return out

    in_maps = []
    for core in range(cfg.n_cores):
        c0 = core * cfg.DC
        ch = slice(c0, c0 + cfg.DC)
        # diag(fD + bD) per channel tile, bf16
        dsum = per["f"]["D"][ch] + per["b"]["D"][ch]
        dsd = np.zeros((P, cfg.CHT, P), f32)
        for c in range(cfg.CHT):
            np.fill_diagonal(dsd[:, c, :],
                             dsum[c * cfg.P_CH:(c + 1) * cfg.P_CH])
        winT = np.concatenate(
            [W_in[ch, :].T, W_in[cfg.DI + c0:cfg.DI + c0 + cfg.DC, :].T],
            axis=1)
        m = {
            "x": x,
            "winT": winT.astype(bf16),
            "wconv": pad_p(
                W_conv[ch].reshape(cfg.CHT, cfg.P_CH, cfg.KC)
                .transpose(1, 0, 2).reshape(cfg.P_CH, cfg.CHT * cfg.KC)),
            "bconv": pad_p(col_layout(b_conv[ch])),
            "dsd": dsd.reshape(P, cfg.CHT * P).astype(bf16),
            "woutT": (W_out.T * 0.5).astype(bf16),
            "ident": ident.astype(bf16),
            "rall": rall_flat.astype(bf16),
            "tsel": tsel_flat.astype(bf16),
            "sall": sall_flat.astype(bf16),
        }
        for d in "fb":
            pd = per[d]
            m[f"wx{d}T"] = pd["Wx"][:, ch].T.astype(bf16)
            m[f"wdt{d}T"] = pd["Wdt"][ch, :].T.astype(bf16)
            m[f"bdt{d}"] = pad_p(col_layout(pd["bdt"][ch]))
            Ac = pd["A"][ch]
            acol = np.empty((P, cfg.NT), f32)
            pidx = np.arange(P)
            for j in range(cfg.NT):
                acol[:, j] = Ac[8 * j + pidx // 16, pidx % 16]
            m[f"acol{d}"] = acol
        in_maps.append({k: np.ascontiguousarray(v) for k, v in m.items()})
    return in_maps


def gather_out(cfg: Cfg, results: list[dict]) -> np.ndarray:
    out = np.concatenate(
        [np.asarray(results[i]["out_rs"]).astype(np.float32)
         for i in range(cfg.n_cores)], axis=0)
    # rows are in pi (interleaved) order: row 2q+b = (batch b, token q)
    return np.ascontiguousarray(
        out.reshape(cfg.L, cfg.B, cfg.M).transpose(1, 0, 2))


def kernel(**inputs) -> np.ndarray:
    cfg = FULL
    from concourse.bass_utils import run_bass_kernel_spmd
    nc = build_program(cfg)
    in_maps = host_prep(cfg, inputs)
    res = run_bass_kernel_spmd(nc, in_maps, core_ids=list(range(cfg.n_cores)))
    return gather_out(cfg, res.results)

